# revision 29
# baseline (speedup 1.0000x reference)
"""Binarized 3x3 conv (N=32, C=256->256, H=W=56, pad 1) on 8 TRN2 NeuronCores.

Sharding: data-parallel over batch (4 images per core), weights replicated.

Math: binarize exactly via
  xb = (x >= 0) - 0.5            in {+-0.5}  (exact in fp8 e4m3)
  wb = (w >= 0) - 0.5            in {+-0.5}
so every product is exactly +-0.25; fp32 PSUM accumulation is exact
(quarter-integer partial sums, |.| <= 576 << 2^22) and the drain scales by
4.0 (exact) to recover the +-1 x +-1 convolution. sign(0)=+1 is honored.
x is cast f32->bf16 during the input DMA (sign-exact for all normal f32;
only |x| < 2^-133 would flush, absent in this input distribution).

Conv as matmul: the padded (58x58) binarized image lives flat in SBUF as
four row-bands (2 row-groups of 8 output rows each + halo; the last band
holds one group), so for each kernel tap (kh,kw) the needed input window is
a CONTIGUOUS span of the flat padded band shifted by (kh-1)*58+(kw-1).
Banding lets the conv start after ~1/4 of the input DMA instead of all of
it. Outputs are computed on the padded grid (464-wide spans = 8 padded
rows); the two garbage columns per row are dropped at drain time.

TensorE: fp8 DoubleRow matmuls contract all 256 input channels in one
instruction (K=128 partitions x 2 interleaved weights/cell), 9 accumulating
taps per output tile. Loop order (band, cc, rg) outer / (tap, n) inner makes
the 4 images' matmuls share one weight load per tap; a post-compile pass
deletes the redundant LDWEIGHTS (identical AP, no sync info) so the PE
sequencer dispatches 126 weight loads instead of 504.

Weights: host-side layout permute to [ci, co, kh, kw] (pure data movement;
binarize stays on device) makes the weight DMA contiguous 9KB runs instead
of a 36B-run gather. One DVE binarize per (cc, ci-half) writes the DoubleRow
layout [tap][two][co] directly.

PSUM: per (cc, rg) tile = 4 banks (one per image), double-buffered (8 banks
total) so the ACT bias+drain of tile i overlaps the matmuls of tile i+1.
"""

import json
import os
os.environ.setdefault("CONCOURSE_SCRUB_NEFF_DEBUG_INFO", "1")

import numpy as np

import concourse.bass as bass
import concourse.mybir as mybir
import concourse.tile as tile
from concourse import bacc, bass_utils

N_CORES = 8
N, CIN, H, W = 32, 256, 56, 56
COUT, KS = 256, 3
NPC = N // N_CORES          # images per core
HP, WP = H + 2, W + 2       # padded spatial (58x58)
LEAD = 64                   # per-band front pad so tap offsets never go negative
NROW_GROUPS = 7
ROWS_PER_GROUP = H // NROW_GROUPS   # 8
FREE = ROWS_PER_GROUP * WP          # 464 <= 512 (one PSUM bank, fp32)
CI_CHUNKS = CIN // 128
CO_CHUNKS = COUT // 128

# row bands: (row groups, first padded row held, padded rows held, images);
# the first band is split by image pair so the first matmuls need only half
# of its input DMA
BANDS = []
for _rgs, _imgs in (((0, 1), (0, 1)), ((0, 1), (2, 3)),
                    ((2, 3), (0, 1, 2, 3)), ((4, 5), (0, 1, 2, 3)),
                    ((6,), (0, 1, 2, 3))):
    _pb = _rgs[0] * ROWS_PER_GROUP
    _nrp = len(_rgs) * ROWS_PER_GROUP + 2
    BANDS.append((_rgs, _pb, _nrp, _imgs))


def _chunk(nrp):  # flat band length: LEAD + padded rows + tail, %16 == 0
    c = LEAD + nrp * WP
    return c + (-c) % 16 + (16 if c % 16 == 0 else 0)


F32 = mybir.dt.float32
BF16 = mybir.dt.bfloat16
FP8 = mybir.dt.float8e4
ALU = mybir.AluOpType
AF = mybir.ActivationFunctionType
DR = mybir.MatmulPerfMode.DoubleRow


def _body(tc, x_d, w_d, b_d, o_d, repeats=1, parts='full'):
    nc = tc.nc

    from contextlib import ExitStack
    ctx = ExitStack()
    with ctx:
        const_pool = ctx.enter_context(tc.tile_pool(name="const", bufs=1))
        wd_pool = ctx.enter_context(tc.tile_pool(name="wd", bufs=1))
        xpad_pool = ctx.enter_context(tc.tile_pool(name="xpad", bufs=1))
        xin_pool = ctx.enter_context(tc.tile_pool(name="xin", bufs=1))
        out_pool = ctx.enter_context(tc.tile_pool(name="outs", bufs=2))

        bias_sb = const_pool.tile([128, CO_CHUNKS], F32, tag="bias", name="bias_sb")
        nc.sync.dma_start(bias_sb[:], b_d.rearrange("(c p) -> p c", p=128))

        # ---- weight phase ----
        # w_d is host-permuted to [ci, co, kh, kw], so the load is contiguous
        # 4608B runs per partition. wd8[cc]: [128 ci_local, 9*256] fp8 {+-.5},
        # free idx = tap*256 + two*128 + co (DoubleRow lhsT layout; per-tap
        # slice [k][two][m], contraction pairs (k, two) on both operands).
        # Loaded in (cc, two) quarters so wd8[0] is ready before wd8[1]'s
        # DMA — the first matmuls only need wd8[0].
        wd8 = []
        for cc in range(CO_CHUNKS):
            wt = wd_pool.tile([128, KS * KS * 256], FP8, tag=f"wd{cc}",
                              name=f"wd8_{cc}")
            wd8.append(wt)
        wstage = ctx.enter_context(tc.tile_pool(name="wstage", bufs=1))

        def emit_w(cc):
            for two in range(CI_CHUNKS):
                # f32 -> bf16 cast during DMA (SWDGE); sign-exact here
                wstg = wstage.tile([128, 128 * KS * KS], BF16,
                                   tag=f"wstg{two}_{cc}",
                                   name=f"wstg{two}_{cc}")
                nc.gpsimd.dma_start(
                    wstg[:],
                    w_d[two * 128:(two + 1) * 128,
                        cc * 128:(cc + 1) * 128]
                    .rearrange("k m kh kw -> k (m kh kw)"))
                dst = wd8[cc][:].rearrange(
                    "k (kp two m) -> k kp two m", two=2, kp=KS * KS
                )[:, :, two, :]
                src = wstg[:].rearrange(
                    "k (m kp) -> k kp m", kp=KS * KS)
                nc.vector.tensor_scalar(
                    dst, src, 0.0, 0.5, op0=ALU.is_ge, op1=ALU.subtract)

        emit_w(0)

        # ---- input + conv phases (repeated `repeats` times for benching) ----
        with tc.tile_pool(name="cpsum", bufs=2, space="PSUM") as cpsum:
            # PE warm-up: the HAM clock gate holds the PE at 1.2 GHz until
            # it has been busy ~3.4us. The real matmuls only start once the
            # first input band lands (~9us), so spend the idle window on
            # zero x zero dummy matmuls into a scratch PSUM slot — the real
            # stream then runs at the full 2.4 GHz from its first
            # instruction. Results are all-zero and discarded.
            warm = const_pool.tile([128, 640], FP8, tag="warm", name="warm")
            nc.vector.memset(warm[:], 0.0)
            wpp = cpsum.tile([128, 512], F32, tag="cps", name="warmpp")
            for _ in range(20):
                nc.tensor.matmul(wpp[:], warm[:, 0:128], warm[:, 128:640],
                                 start=True, stop=True)
            for rep in range(repeats):
                hist = {}
                for bi, (rgs, pb, nrp, imgs) in enumerate(BANDS):
                    ck = _chunk(nrp)
                    ni = len(imgs)
                    n0 = imgs[0]
                    # input rows this band binarizes (global padded rows
                    # [max(pb,1), min(pb+nrp-1, HP-2)] are data rows)
                    gp0, gp1 = max(pb, 1), min(pb + nrp - 1, HP - 2)
                    r0, r1 = gp0 - 1, gp1 - 1          # input row span
                    tr0 = gp0 - pb                      # band-local row
                    nrows = r1 - r0 + 1

                    # bands after the first row level re-use the 2 halo rows
                    # already staged by the previous band instead of
                    # re-fetching them from HBM
                    halo = 2 if bi >= 2 else 0
                    f_r0 = r0 + halo
                    f_nrows = r1 - f_r0 + 1

                    xr = []
                    for two in range(CI_CHUNKS):
                        x_raw = xin_pool.tile(
                            [128, ni * f_nrows * W], BF16,
                            tag=f"xraw{bi}_{two}", name=f"xraw{rep}_{bi}_{two}")
                        xr.append(x_raw)
                        # f32 -> bf16 cast during DMA (SWDGE); sign-exact here
                        nc.gpsimd.dma_start(
                            x_raw[:].rearrange("c (n s) -> c n s", n=ni),
                            x_d[n0:n0 + ni, two * 128:(two + 1) * 128,
                                f_r0:r1 + 1]
                            .rearrange("n c h w -> c n (h w)"))
                    hist[bi] = (xr, f_r0)

                    xb = xpad_pool.tile([128, ni * 2 * ck], FP8,
                                        tag=f"xb{bi}", name=f"xb{rep}_{bi}")
                    xg = xb[:].rearrange("c (g s) -> c g s", s=ck)
                    nc.vector.memset(xg[:, :, 0:LEAD], 0.0)
                    nc.vector.memset(xg[:, :, LEAD + nrp * WP:ck], 0.0)
                    xgrid = xg[:, :, LEAD:LEAD + nrp * WP] \
                        .rearrange("c g (h w) -> c g h w", w=WP)
                    if pb == 0:
                        nc.vector.memset(xgrid[:, :, 0:1, :], 0.0)
                    if pb + nrp == HP:
                        nc.vector.memset(xgrid[:, :, nrp - 1:nrp, :], 0.0)
                    nc.vector.memset(xgrid[:, :, :, 0:1], 0.0)
                    nc.vector.memset(xgrid[:, :, :, WP - 1:WP], 0.0)
                    def xb_rows(t_two, ln0, ln1, row0, nrx):
                        return xb[:].rearrange(
                            "c (n t s) -> c n t s", t=2, s=ck
                        )[:, ln0:ln1, t_two, LEAD:LEAD + nrp * WP] \
                            .rearrange("c n (h w) -> c n h w", w=WP
                                       )[:, :, row0:row0 + nrx, 1:W + 1]

                    for two in range(CI_CHUNKS):
                        nc.vector.tensor_scalar(
                            xb_rows(two, 0, ni, tr0 + halo, f_nrows),
                            xr[two][:].rearrange("c (n h w) -> c n h w",
                                                 n=ni, w=W),
                            0.0, 0.5, op0=ALU.is_ge, op1=ALU.subtract)
                        if halo:
                            srcs = ([(hist[0], 0, 2), (hist[1], 2, 4)]
                                    if bi == 2 else [(hist[bi - 1], 0, ni)])
                            for (sxr, sf_r0), l0, l1 in srcs:
                                sv = sxr[two][:].rearrange(
                                    "c (n h w) -> c n h w", n=l1 - l0, w=W)
                                nc.vector.tensor_scalar(
                                    xb_rows(two, l0, l1, tr0, halo),
                                    sv[:, :, r0 - sf_r0:r0 - sf_r0 + halo, :],
                                    0.0, 0.5, op0=ALU.is_ge, op1=ALU.subtract)
                    xp = [xb[:, ln * 2 * ck:(ln + 1) * 2 * ck]
                          for ln in range(ni)]

                    if rep == 0 and bi == 0:
                        emit_w(1)
                    if parts == 'input':
                        continue
                    for cc in range(CO_CHUNKS):
                        ob = out_pool.tile(
                            [128, ni * len(rgs) * ROWS_PER_GROUP * W],
                            F32, tag=f"ob{cc}", name=f"ob{rep}_{bi}_{cc}")
                        for rl, rg in enumerate(rgs):
                            pp = cpsum.tile([128, ni * 512], F32,
                                            tag="cps",
                                            name=f"cps{rep}_{bi}_{cc}_{rl}")
                            for kpos in range(KS * KS):
                                kh, kw = divmod(kpos, KS)
                                lhsT = wd8[cc][:, kpos * 256:(kpos + 1) * 256] \
                                    .rearrange("k (two m) -> k two m", two=2)
                                off = (LEAD + WP + rl * FREE
                                       + (kh - 1) * WP + (kw - 1))
                                for ln in range(ni):
                                    rhs = xp[ln].rearrange(
                                        "k (two s) -> k two s",
                                        s=ck)[:, :, off:off + FREE]
                                    nc.tensor.matmul(
                                        pp[:, ln * 512:ln * 512 + FREE],
                                        lhsT, rhs,
                                        start=(kpos == 0),
                                        stop=(kpos == KS * KS - 1),
                                        perf_mode=DR)
                            # drain in image halves so the first half's
                            # output DMA can overlap the second's drain
                            halves = ([(0, 2), (2, 4)] if ni == 4
                                      else [(0, ni)])
                            for h0, h1 in halves:
                                drain_in = pp[:].rearrange(
                                    "m (n s) -> m n s", n=ni
                                )[:, h0:h1, :FREE] \
                                    .rearrange("m n (r c) -> m n r c", c=WP
                                               )[:, :, :, 1:W + 1]
                                drain_out = ob[:].rearrange(
                                    "m (n r c) -> m n r c", n=ni, c=W
                                )[:, h0:h1,
                                  rl * ROWS_PER_GROUP:(rl + 1) * ROWS_PER_GROUP, :]
                                nc.scalar.activation(
                                    drain_out, drain_in,
                                    AF.Identity, bias=bias_sb[:, cc:cc + 1],
                                    scale=4.0)
                        if parts != 'nooutdma':
                            nr = len(rgs) * ROWS_PER_GROUP
                            for h0, h1 in halves:
                                dst = o_d[n0 + h0:n0 + h1,
                                          cc * 128:(cc + 1) * 128,
                                          rgs[0] * ROWS_PER_GROUP:
                                          rgs[0] * ROWS_PER_GROUP + nr, :] \
                                    .rearrange("n c r w -> c n r w")
                                src = ob[:].rearrange(
                                    "m (n r c) -> m n r c", n=ni, c=W
                                )[:, h0:h1, :, :]
                                nc.sync.dma_start(dst, src)


def _dedup_ldweights(m):
    """Delete InstLdweights that are byte-identical to the immediately
    preceding InstLdweights in the same block with no sync info — the PE
    array already holds those weights, so the reload is pure overhead.
    Conservative: any unrecognized PE-array-writing instruction resets
    the tracked state. Operates on the BIR json and returns a new Module.
    """
    j = json.loads(mybir.module_to_json_string(m))
    deleted = []
    for fn in j.get("functions", []):
        for bb in fn.get("basicblocks") or fn.get("blocks") or []:
            insts = bb.get("instructions", [])
            keep = []
            last_sig = None
            for inst in insts:
                op = inst.get("opcode") or ""
                if op == "Ldweights":
                    si = inst.get("sync_info") or {}
                    waits = si.get("on_wait") or []
                    upds = si.get("on_update") or []
                    sig = json.dumps(
                        [inst.get("ins"), inst.get("perf_mode"),
                         inst.get("is_transpose"), inst.get("engine"),
                         inst.get("tile_position")], sort_keys=True)
                    if sig == last_sig and not waits and not upds:
                        deleted.append(inst.get("name"))
                        continue
                    last_sig = sig
                elif op == "Matmult":
                    if inst.get("ldweights") not in (False, None):
                        last_sig = None
                elif op == "MatmultMx":
                    last_sig = None
                keep.append(inst)
            bb["instructions"] = keep
    if not deleted:
        return m, 0
    dead = set(deleted)
    # scrub dangling references to deleted names from dependency metadata
    def scrub(obj):
        if isinstance(obj, dict):
            for k, v in list(obj.items()):
                if isinstance(v, list) and v and all(
                        isinstance(x, str) for x in v):
                    if any(x in dead for x in v):
                        obj[k] = [x for x in v if x not in dead]
                else:
                    scrub(v)
        elif isinstance(obj, list):
            for v in obj:
                scrub(v)
    scrub(j)
    return mybir.module_from_json_string(json.dumps(j)), len(deleted)


_nc_cache = {}


def _get_nc(repeats=1, parts='full'):
    key = (repeats, parts)
    if key not in _nc_cache:
        nc = bacc.Bacc("TRN2", debug=False)
        x_d = nc.dram_tensor("x", [NPC, CIN, H, W], F32, kind="ExternalInput").ap()
        w_d = nc.dram_tensor("w", [CIN, COUT, KS, KS], F32,
                             kind="ExternalInput").ap()
        b_d = nc.dram_tensor("b", [COUT], F32, kind="ExternalInput").ap()
        o_d = nc.dram_tensor("out", [NPC, COUT, H, W], F32,
                             kind="ExternalOutput").ap()
        with tile.TileContext(nc) as tc:
            _body(tc, x_d, w_d, b_d, o_d, repeats=repeats, parts=parts)
        nc.compile()
        nc.m, _ndel = _dedup_ldweights(nc.m)
        _nc_cache[key] = nc
    return _nc_cache[key]


def _run(inputs, repeats=1, **kwargs):
    x, w, b = inputs["x"], inputs["w"], inputs["b"]
    assert x.shape == (N, CIN, H, W), x.shape
    nc = _get_nc(repeats)
    # host-side layout permute only (binarize happens on device)
    w_t = np.ascontiguousarray(
        np.asarray(w, dtype=np.float32).transpose(1, 0, 2, 3))
    b_c = np.ascontiguousarray(b, dtype=np.float32)
    in_maps = [{
        "x": np.ascontiguousarray(x[i * NPC:(i + 1) * NPC], dtype=np.float32),
        "w": w_t,
        "b": b_c,
    } for i in range(N_CORES)]
    res = bass_utils.run_bass_kernel_spmd(
        nc, in_maps, core_ids=list(range(N_CORES)), **kwargs)
    out = np.concatenate([res.results[i]["out"] for i in range(N_CORES)], axis=0)
    return out, res


def kernel(**inputs) -> np.ndarray:
    out, _ = _run(inputs)
    return out
